# revision 34
# baseline (speedup 1.0000x reference)
"""CorefHead Trainium2 kernel.

Reference computation (B=64, S=512, H=1024, HID=512):
  emb_a = span_mean(bert, offsets[:,0:2])   # [B,H]
  emb_b = span_mean(bert, offsets[:,2:4])   # [B,H]
  emb_p = bert[b, offsets[:,4]]             # [B,H]
  x = concat([emb_a, emb_b, emb_p], -1)     # [B,3H]
  h = leaky_relu(batchnorm_eval(x @ W1 + b1), 0.01)
  out = h @ W2 + b2                         # [B,3]

Strategy: pure data parallel, batch sharded 8 ways (8 batches/core),
DMA-volume minimized:
  - Host ships only the exact union rows (span A + span B) per batch,
    packed back-to-back across the core's 8 batches into 128-row chunks
    (chunks may cross batch boundaries). Rows are fp8-e4m3: span means
    average ~170 rows and the pron row dominates the final signal, so
    fp8 noise on span rows stays ~0.5% at the output. The pron rows ship
    separately in fp32 and are transposed on the PE.
  - mm1 (PE): per (DMA group, h-chunk) a PSUM tile [128, 16] accumulates
    bert_chunk.T @ mask_chunk, flushed into an SBUF accumulator by the
    DVE (PSUM accumulation groups must close before the next opens);
    the mask column encodes (span, slot) so batch identity lives in the
    mask and chunks may mix batches freely.
  - mm2 (PE, swapped operands): per hid quarter q, phT[q] +=
    W1sub[128k, 128hid].T @ xT[128k, 8] -> h transposed directly (no
    on-device transpose of h). Per-q rotating PSUM tiles + interleaved
    BN+LeakyReLU (DVE) and mm3 (PE, out[3,8] accumulated in SBUF) let
    the DVE consume quarter q while the PE runs quarter q+1.
  - DMA: bert rides the SP ring in ~0.5 MB groups (first group small to
    prime the mm1 pipeline); consts + W1 ride the ACT ring; W1 is only
    needed by mm2 at the end so bert is never stuck behind it.
Host gathers per-core [3, 8] outputs and undoes the batch permutation.
"""

import numpy as np

B, S, H = 64, 512, 1024
HID = 512
EPS = 1e-5
NCORES = 8
BPC = B // NCORES  # batches per core
KC = 3 * H // 128  # 24 contraction chunks for mm2
HC = H // 128      # 8 h-chunks per embedding
NQ = HID // 128    # 4 hid quarters

# bert span rows + masks in fp8-e4m3 (halves DMA vs bf16); pron fp32.
BERT_FP8 = True
# Spans shorter than this get a second pass of fp8 residual rows
# (v - fp8(v), same mask column): short spans don't average away fp8
# noise, and two fp8 levels beat bf16 precision for ~5% extra rows.
LTHR = 32
# W1 (and the mm2 xT operand) in bf16.
W1_BF16 = True
# W1 span blocks (rows 0..2047) + the span xT columns in fp8: their
# error contribution to h is attenuated ~10x because the pron block
# dominates h's variance. The pron block of W1 stays bf16.
W1_SPAN_FP8 = False

# Test-harness hooks (harness calls kernel() with TRACE=False default).
TRACE = False
LAST_RESULT = None

_PROGRAM_CACHE: dict = {}


def _bert_groups(totch: int):
    """Chunk-group sizes for the bert DMA: small first group to prime
    the mm1 pipeline, ~4-chunk (512 KB fp8) transfers in the middle,
    small last group so mm1 drains right at DMA end."""
    if totch <= 2:
        return [totch]
    groups = [2]
    left = totch - 2
    while left > 0:
        g = min(4, left)
        groups.append(g)
        left -= g
    if groups[-1] > 2:
        groups[-1] -= 1
        groups.append(1)
    return groups


def _build_program(totch: int):
    import concourse.bacc as bacc
    import concourse.tile as tile
    import concourse.mybir as mybir
    from concourse.bass import MemorySpace
    from concourse.masks import make_identity

    f32 = mybir.dt.float32
    bdt = mybir.dt.float8e4 if BERT_FP8 else mybir.dt.bfloat16
    wdt = mybir.dt.bfloat16 if W1_BF16 else f32
    sdt = mybir.dt.float8e4 if W1_SPAN_FP8 else wdt

    nc = bacc.Bacc("TRN2", target_bir_lowering=False, debug=False,
                   num_devices=NCORES)

    NMC = 2 * BPC  # mask columns: (span e, slot) -> e*BPC + slot

    bert_d = nc.dram_tensor("bertp", [128, totch * H], bdt,
                            kind="ExternalInput").ap()
    mask_d = nc.dram_tensor("maskp", [128, totch, NMC], bdt,
                            kind="ExternalInput").ap()
    sfac_d = nc.dram_tensor("sfac", [128, NMC], f32, kind="ExternalInput").ap()
    pron_d = nc.dram_tensor("pron", [BPC, H], f32, kind="ExternalInput").ap()
    w1s_d = nc.dram_tensor("w1sP", [128, 2 * HC, HID], sdt,
                           kind="ExternalInput").ap()
    w1p_d = nc.dram_tensor("w1pP", [128, HC, HID], wdt,
                           kind="ExternalInput").ap()
    bnb_d = nc.dram_tensor("bnbP", [128, NQ], f32, kind="ExternalInput").ap()
    w2_d = nc.dram_tensor("w2P", [128, NQ, 3], f32, kind="ExternalInput").ap()
    b2_d = nc.dram_tensor("b2c", [3, 1], f32, kind="ExternalInput").ap()
    out_d = nc.dram_tensor("out", [3, BPC], f32, kind="ExternalOutput").ap()

    groups = _bert_groups(totch)

    with tile.TileContext(nc) as tc:
        with (
            tc.tile_pool(name="singles", bufs=1) as singles,
            tc.tile_pool(name="head", bufs=1) as head,
            tc.tile_pool(name="psum_x", bufs=2, space=MemorySpace.PSUM) as psum_x,
            tc.tile_pool(name="psum_p", bufs=1, space=MemorySpace.PSUM) as psum_p,
            tc.tile_pool(name="psum_hs", bufs=2, space=MemorySpace.PSUM) as psum_hs,
            tc.tile_pool(name="psum_hp", bufs=2, space=MemorySpace.PSUM) as psum_hp,
        ):
            # --- consts on the ACT ring (tiny, land first) ---
            mask_t = singles.tile([128, totch, NMC], bdt)
            nc.scalar.dma_start(out=mask_t, in_=mask_d)
            sfac_t = singles.tile([128, NMC], f32)
            nc.scalar.dma_start(out=sfac_t, in_=sfac_d)
            pron_t = singles.tile([BPC, H], f32)
            nc.scalar.dma_start(out=pron_t, in_=pron_d)
            bnb_t = head.tile([128, NQ], f32)
            nc.scalar.dma_start(out=bnb_t, in_=bnb_d)
            w2_t = head.tile([128, NQ, 3], f32)
            nc.scalar.dma_start(out=w2_t, in_=w2_d)
            b2_t = head.tile([3, 1], f32)
            nc.scalar.dma_start(out=b2_t, in_=b2_d)
            idt = singles.tile([BPC, BPC], f32)
            make_identity(nc, idt)

            # --- bert row chunks on the SP ring, grouped ---
            bt = singles.tile([128, totch * H], bdt)
            c0 = 0
            for g in groups:
                nc.sync.dma_start(out=bt[:, c0 * H:(c0 + g) * H],
                                  in_=bert_d[:, c0 * H:(c0 + g) * H])
                c0 += g

            # --- W1 on the ACT ring, after consts (needed only by mm2) ---
            w1p_t = singles.tile([128, HC, HID], wdt)
            for i in range(2):
                half = HC // 2
                nc.scalar.dma_start(out=w1p_t[:, half * i:half * (i + 1), :],
                                    in_=w1p_d[:, half * i:half * (i + 1), :])
            w1s_t = singles.tile([128, 2 * HC, HID], sdt)
            for i in range(2):
                nc.scalar.dma_start(out=w1s_t[:, HC * i:HC * (i + 1), :],
                                    in_=w1s_d[:, HC * i:HC * (i + 1), :])

            # xT split: span cols (e*BPC + slot, e in {A, B}) and pron
            xTs_t = singles.tile([128, HC, 2 * BPC], sdt)
            xTp_t = singles.tile([128, HC, BPC], wdt)

            # --- pron embedding: fp32 rows, transposed via PE ---
            for hc in range(HC):
                pxp = psum_p.tile([128, BPC], f32, tag="pxp")
                nc.tensor.transpose(pxp, pron_t[:, hc * 128:(hc + 1) * 128],
                                    idt)
                nc.vector.tensor_copy(xTp_t[:, hc, :], pxp)

            # --- mm1: span sums for all slots at once ---
            # The mask column encodes (span, slot), so chunks may mix
            # batches freely. PSUM accumulation groups must be closed
            # before the next opens (concurrent open groups in a bank
            # corrupt each other), so accumulate per DMA group in PSUM
            # and flush to an SBUF accumulator.
            xacc = singles.tile([128, HC, NMC], f32)
            hp_sb = singles.tile([128, NQ, BPC], f32)
            c0 = 0
            for gi, g in enumerate(groups):
                if gi == len(groups) - 1:
                    # pron half of mm2: xTp and w1p are ready by now, so
                    # this fills the PE while the last bert group lands.
                    for q in range(NQ):
                        phTp = psum_hp.tile([128, BPC], f32, tag="phTp")
                        for hc in range(HC):
                            nc.tensor.matmul(
                                phTp,
                                w1p_t[:, hc, q * 128:(q + 1) * 128],
                                xTp_t[:, hc, :],
                                start=(hc == 0), stop=(hc == HC - 1))
                        nc.vector.tensor_copy(hp_sb[:, q, :], phTp)
                for hc in range(HC):
                    pxg = psum_x.tile([128, NMC], f32, tag="px")
                    for j in range(g):
                        sc = c0 + j
                        nc.tensor.matmul(
                            pxg,
                            bt[:, sc * H + hc * 128:sc * H + (hc + 1) * 128],
                            mask_t[:, sc, :],
                            start=(j == 0),
                            stop=(j == g - 1),
                        )
                    if gi == 0:
                        nc.vector.tensor_copy(xacc[:, hc, :], pxg)
                    else:
                        nc.vector.tensor_add(xacc[:, hc, :],
                                             xacc[:, hc, :], pxg)
                c0 += g
            # fp32 scale by 1/span_len on the SBUF->SBUF copy
            for hc in range(HC):
                nc.vector.tensor_mul(xTs_t[:, hc, :], xacc[:, hc, :],
                                     sfac_t)

            # --- mm2: hT[q] = sum_kc W1sub.T @ xT chunk (24 k-chunks) ---
            # mm2 -> BN+LeakyReLU -> mm3, interleaved per hid quarter:
            # per-q PSUM tiles (rotating banks) let the DVE consume
            # quarter q while the PE runs quarter q+1. The span part
            # (scaled if fp8) and pron part accumulate separately and
            # recombine on the DVE; mm3 results accumulate in SBUF so
            # no PSUM group stays open across quarters.
            SPAN_UNSCALE = 1.0 / (64.0 * 32.0) if W1_SPAN_FP8 else 1.0
            o_acc = head.tile([3, BPC], f32)

            def mm2_span_a(q, phTs):
                # kc 0..7 needs only the first w1s transfer + xTs, so
                # for q<2 it runs while the last w1s bytes stream in.
                for kc in range(HC):
                    nc.tensor.matmul(
                        phTs,
                        w1s_t[:, kc, q * 128:(q + 1) * 128],
                        xTs_t[:, kc, 0:BPC],
                        start=(kc == 0), stop=(kc == HC - 1))

            phTs_q = {}
            for q in range(2):
                phTs = psum_hs.tile([128, BPC], f32, tag="phTs")
                phTs_q[q] = phTs
                mm2_span_a(q, phTs)
            for q in range(NQ):
                if q in phTs_q:
                    phTs = phTs_q[q]
                else:
                    phTs = psum_hs.tile([128, BPC], f32, tag="phTs")
                    mm2_span_a(q, phTs)
                # span-B half continues the same PSUM accumulation
                # (stop only affects sim bookkeeping, not hardware).
                for kc in range(HC, 2 * HC):
                    nc.tensor.matmul(
                        phTs,
                        w1s_t[:, kc, q * 128:(q + 1) * 128],
                        xTs_t[:, kc - HC, BPC:2 * BPC],
                        start=False, stop=(kc == 2 * HC - 1),
                        skip_group_check=True)
                t0_t = head.tile([128, BPC], f32, tag=f"t0_{q % 2}")
                nc.vector.tensor_scalar_add(t0_t, hp_sb[:, q, :],
                                            bnb_t[:, q:q + 1])
                t_t = head.tile([128, BPC], f32, tag=f"t_{q % 2}")
                nc.vector.scalar_tensor_tensor(
                    t_t, phTs, SPAN_UNSCALE, t0_t,
                    op0=mybir.AluOpType.mult, op1=mybir.AluOpType.add)
                y_t = head.tile([128, BPC], f32, tag=f"y_{q % 2}")
                # y = max(0.01 * t, t)
                nc.vector.scalar_tensor_tensor(
                    y_t, t_t, 0.01, t_t,
                    op0=mybir.AluOpType.mult, op1=mybir.AluOpType.max)
                pm3 = psum_p.tile([3, BPC], f32, tag="m3")
                nc.tensor.matmul(pm3, w2_t[:, q, :], y_t,
                                 start=True, stop=True)
                if q == 0:
                    nc.vector.tensor_scalar_add(o_acc, pm3, b2_t)
                else:
                    nc.vector.tensor_add(o_acc, o_acc, pm3)
            nc.sync.dma_start(out=out_d, in_=o_acc)

    nc.compile()
    return nc


def _prep_core_inputs(bert8, bert_f32, offsets, batch_idx, totch, np8):
    """Build the per-core input map for the given batch indices."""
    NMC = 2 * BPC
    bertp = np.zeros((totch, 128, H), dtype=bert8.dtype)
    sfac = np.ones((NMC,), dtype=np.float32)
    pron = np.empty((BPC, H), dtype=np.float32)
    flat_bert = bertp.reshape(totch * 128, H)
    flat_mask = np.zeros((totch * 128, NMC), dtype=bert8.dtype)
    rmw = np.asarray(1.0 / 64.0, dtype=np8)  # exact in fp8
    row = 0
    for slot, gb in enumerate(batch_idx):
        a0, a1, b0, b1_, p = (int(v) for v in offsets[gb])
        rows = np.union1d(np.arange(a0, a1 + 1), np.arange(b0, b1_ + 1))
        n = rows.shape[0]
        flat_bert[row:row + n] = bert8[gb, rows]
        flat_mask[row:row + n, 0 * BPC + slot] = \
            ((rows >= a0) & (rows <= a1)).astype(np.float32).astype(np8)
        flat_mask[row:row + n, 1 * BPC + slot] = \
            ((rows >= b0) & (rows <= b1_)).astype(np.float32).astype(np8)
        xsc = 32.0 if W1_SPAN_FP8 else 1.0
        sfac[0 * BPC + slot] = xsc / (a1 - a0 + 1)
        sfac[1 * BPC + slot] = xsc / (b1_ - b0 + 1)
        pron[slot] = bert_f32[gb, p]
        row += n
        if BERT_FP8:
            for e, (s0, s1) in enumerate(((a0, a1), (b0, b1_))):
                L = s1 - s0 + 1
                if L >= LTHR:
                    continue
                res = (bert_f32[gb, s0:s1 + 1]
                       - bert8[gb, s0:s1 + 1].astype(np.float32))
                flat_bert[row:row + L] = (res * 64.0).astype(np8)
                flat_mask[row:row + L, e * BPC + slot] = rmw
                row += L
    # partition-major layout: each SBUF partition line is contiguous DRAM
    maskp = np.ascontiguousarray(flat_mask.reshape(totch, 128, NMC)
                                 .transpose(1, 0, 2))
    return {
        "bertp": np.ascontiguousarray(
            bertp.transpose(1, 0, 2).reshape(128, totch * H)),
        "maskp": maskp,
        "sfac": np.broadcast_to(sfac, (128, NMC)).copy(),
        "pron": pron,
    }


def kernel(bert_outputs, offsets, W1, b1, gamma, beta, running_mean,
           running_var, W2, b2):
    import ml_dtypes

    np8 = ml_dtypes.float8_e4m3 if BERT_FP8 else ml_dtypes.bfloat16

    bert_f32 = np.ascontiguousarray(np.asarray(bert_outputs, dtype=np.float32))
    bert8 = bert_f32.astype(np8)
    offs = np.asarray(offsets).astype(np.int64)
    W1 = np.asarray(W1, dtype=np.float32)
    b1 = np.asarray(b1, dtype=np.float32)
    gamma = np.asarray(gamma, dtype=np.float32)
    beta = np.asarray(beta, dtype=np.float32)
    rm = np.asarray(running_mean, dtype=np.float32)
    rv = np.asarray(running_var, dtype=np.float32)
    W2 = np.asarray(W2, dtype=np.float32)
    b2 = np.asarray(b2, dtype=np.float32)

    # Fold BN eval-mode stats: bn(xW1 + b1) = x(W1*s) + ((b1 - mean)*s + beta)
    s = gamma / np.sqrt(rv + EPS)
    bias = (b1 - rm) * s + beta
    W1s = W1 * s[None, :]
    sdt_np = ml_dtypes.float8_e4m3 if W1_SPAN_FP8 else (
        ml_dtypes.bfloat16 if W1_BF16 else np.float32)
    wdt_np = ml_dtypes.bfloat16 if W1_BF16 else np.float32
    w1sP = np.ascontiguousarray(
        (W1s[:2 * H] * (64.0 if W1_SPAN_FP8 else 1.0))
        .reshape(2 * HC, 128, HID).transpose(1, 0, 2)
    ).astype(sdt_np)
    w1pP = np.ascontiguousarray(
        W1s[2 * H:].reshape(HC, 128, HID).transpose(1, 0, 2)
    ).astype(wdt_np)
    bnbP = np.ascontiguousarray(bias.reshape(NQ, 128).T)
    w2P = np.ascontiguousarray(W2.reshape(NQ, 128, 3).transpose(1, 0, 2))
    b2c = np.ascontiguousarray(b2.reshape(3, 1))

    # Exact union rows per batch; balance total rows across cores (LPT,
    # exactly BPC batches per core).
    nrows = np.empty(B, dtype=np.int64)
    for gb in range(B):
        a0, a1, b0, b1_, _ = (int(v) for v in offs[gb])
        # union size without materializing: overlap or disjoint
        if b0 <= a1 and a0 <= b1_:
            nrows[gb] = max(a1, b1_) - min(a0, b0) + 1
        else:
            nrows[gb] = (a1 - a0 + 1) + (b1_ - b0 + 1)
        if BERT_FP8:
            for s0, s1 in ((a0, a1), (b0, b1_)):
                if s1 - s0 + 1 < LTHR:
                    nrows[gb] += s1 - s0 + 1
    order = np.argsort(-nrows, kind="stable")
    loads = np.zeros(NCORES, dtype=np.int64)
    counts = np.zeros(NCORES, dtype=np.int64)
    asg = [[] for _ in range(NCORES)]
    for gb in order:
        open_cores = np.flatnonzero(counts < BPC)
        c = open_cores[np.argmin(loads[open_cores])]
        asg[c].append(int(gb))
        loads[c] += nrows[gb]
        counts[c] += 1
    totch = int((loads.max() + 127) // 128)

    if totch not in _PROGRAM_CACHE:
        _PROGRAM_CACHE[totch] = _build_program(totch)
    nc = _PROGRAM_CACHE[totch]

    shared = {"w1sP": w1sP, "w1pP": w1pP, "bnbP": bnbP, "w2P": w2P,
              "b2c": b2c}
    in_maps = []
    for c in range(NCORES):
        m = _prep_core_inputs(bert8, bert_f32, offs, asg[c], totch, np8)
        m.update(shared)
        in_maps.append(m)

    from concourse import bass_utils
    kwargs = {}
    if TRACE:
        kwargs = {"trace": True, "trace_cores": list(range(NCORES))}
    res = bass_utils.run_bass_kernel_spmd(nc, in_maps,
                                          core_ids=list(range(NCORES)),
                                          **kwargs)
    global LAST_RESULT
    LAST_RESULT = res

    out = np.empty((B, 3), dtype=np.float32)
    for c in range(NCORES):
        out[asg[c]] = res.results[c]["out"].T
    return out


# revision 36
# speedup vs baseline: 1.0062x; 1.0062x over previous
"""CorefHead Trainium2 kernel.

Reference computation (B=64, S=512, H=1024, HID=512):
  emb_a = span_mean(bert, offsets[:,0:2])   # [B,H]
  emb_b = span_mean(bert, offsets[:,2:4])   # [B,H]
  emb_p = bert[b, offsets[:,4]]             # [B,H]
  x = concat([emb_a, emb_b, emb_p], -1)     # [B,3H]
  h = leaky_relu(batchnorm_eval(x @ W1 + b1), 0.01)
  out = h @ W2 + b2                         # [B,3]

Strategy: pure data parallel, batch sharded 8 ways (8 batches/core),
DMA-volume minimized:
  - Host ships only the exact union rows (span A + span B) per batch,
    packed back-to-back across the core's 8 batches into 128-row chunks
    (chunks may cross batch boundaries). Rows are fp8-e4m3: span means
    average ~170 rows and the pron row dominates the final signal, so
    fp8 noise on span rows stays ~0.5% at the output. The pron rows ship
    separately in fp32 and are transposed on the PE.
  - mm1 (PE): per (DMA group, h-chunk) a PSUM tile [128, 16] accumulates
    bert_chunk.T @ mask_chunk, flushed into an SBUF accumulator by the
    DVE (PSUM accumulation groups must close before the next opens);
    the mask column encodes (span, slot) so batch identity lives in the
    mask and chunks may mix batches freely.
  - mm2 (PE, swapped operands): per hid quarter q, phT[q] +=
    W1sub[128k, 128hid].T @ xT[128k, 8] -> h transposed directly (no
    on-device transpose of h). Per-q rotating PSUM tiles + interleaved
    BN+LeakyReLU (DVE) and mm3 (PE, out[3,8] accumulated in SBUF) let
    the DVE consume quarter q while the PE runs quarter q+1.
  - DMA: bert rides the SP ring in ~0.5 MB groups (first group small to
    prime the mm1 pipeline); consts + W1 ride the ACT ring; W1 is only
    needed by mm2 at the end so bert is never stuck behind it.
Host gathers per-core [3, 8] outputs and undoes the batch permutation.
"""

import numpy as np

B, S, H = 64, 512, 1024
HID = 512
EPS = 1e-5
NCORES = 8
BPC = B // NCORES  # batches per core
KC = 3 * H // 128  # 24 contraction chunks for mm2
HC = H // 128      # 8 h-chunks per embedding
NQ = HID // 128    # 4 hid quarters

# bert span rows + masks in fp8-e4m3 (halves DMA vs bf16); pron fp32.
BERT_FP8 = True
# Spans shorter than this get a second pass of fp8 residual rows
# (v - fp8(v), same mask column): short spans don't average away fp8
# noise, and two fp8 levels beat bf16 precision for ~5% extra rows.
LTHR = 32
# W1 (and the mm2 xT operand) in bf16.
W1_BF16 = True
# W1 span blocks (rows 0..2047) + the span xT columns in fp8: their
# error contribution to h is attenuated ~10x because the pron block
# dominates h's variance. The pron block of W1 stays bf16.
W1_SPAN_FP8 = False

# Test-harness hooks (harness calls kernel() with TRACE=False default).
TRACE = False
LAST_RESULT = None

_PROGRAM_CACHE: dict = {}


def _bert_groups(totch: int):
    """Chunk-group sizes for the bert DMA: small first group to prime
    the mm1 pipeline, ~4-chunk (512 KB fp8) transfers in the middle,
    small last group so mm1 drains right at DMA end."""
    if totch <= 2:
        return [totch]
    groups = [2]
    left = totch - 2
    while left > 0:
        g = min(4, left)
        groups.append(g)
        left -= g
    if groups[-1] > 2:
        groups[-1] -= 1
        groups.append(1)
    return groups


def _build_program(totch: int):
    import concourse.bacc as bacc
    import concourse.tile as tile
    import concourse.mybir as mybir
    from concourse.bass import MemorySpace
    from concourse.masks import make_identity

    f32 = mybir.dt.float32
    bdt = mybir.dt.float8e4 if BERT_FP8 else mybir.dt.bfloat16
    wdt = mybir.dt.bfloat16 if W1_BF16 else f32
    sdt = mybir.dt.float8e4 if W1_SPAN_FP8 else wdt

    nc = bacc.Bacc("TRN2", target_bir_lowering=False, debug=False,
                   num_devices=NCORES)

    NMC = 2 * BPC  # mask columns: (span e, slot) -> e*BPC + slot

    bert_d = nc.dram_tensor("bertp", [128, totch * H], bdt,
                            kind="ExternalInput").ap()
    mask_d = nc.dram_tensor("maskp", [128, totch, NMC], bdt,
                            kind="ExternalInput").ap()
    sfac_d = nc.dram_tensor("sfac", [128, NMC], f32, kind="ExternalInput").ap()
    pron_d = nc.dram_tensor("pron", [BPC, H], f32, kind="ExternalInput").ap()
    w1s_d = nc.dram_tensor("w1sP", [128, 2 * HC, HID], sdt,
                           kind="ExternalInput").ap()
    w1p_d = nc.dram_tensor("w1pP", [128, HC, HID], wdt,
                           kind="ExternalInput").ap()
    bnb_d = nc.dram_tensor("bnbP", [128, NQ], f32, kind="ExternalInput").ap()
    w2_d = nc.dram_tensor("w2P", [128, NQ, 3], f32, kind="ExternalInput").ap()
    b2_d = nc.dram_tensor("b2c", [3, 1], f32, kind="ExternalInput").ap()
    out_d = nc.dram_tensor("out", [3, BPC], f32, kind="ExternalOutput").ap()

    groups = _bert_groups(totch)

    with tile.TileContext(nc) as tc:
        with (
            tc.tile_pool(name="singles", bufs=1) as singles,
            tc.tile_pool(name="head", bufs=1) as head,
            tc.tile_pool(name="psum_x", bufs=3, space=MemorySpace.PSUM) as psum_x,
            tc.tile_pool(name="psum_p", bufs=1, space=MemorySpace.PSUM) as psum_p,
            tc.tile_pool(name="psum_hs", bufs=2, space=MemorySpace.PSUM) as psum_hs,
            tc.tile_pool(name="psum_hp", bufs=1, space=MemorySpace.PSUM) as psum_hp,
        ):
            # --- consts on the ACT ring (tiny, land first) ---
            mask_t = singles.tile([128, totch, NMC], bdt)
            nc.scalar.dma_start(out=mask_t, in_=mask_d)
            sfac_t = singles.tile([128, NMC], f32)
            nc.scalar.dma_start(out=sfac_t, in_=sfac_d)
            pron_t = singles.tile([BPC, H], f32)
            nc.scalar.dma_start(out=pron_t, in_=pron_d)
            bnb_t = head.tile([128, NQ], f32)
            nc.scalar.dma_start(out=bnb_t, in_=bnb_d)
            w2_t = head.tile([128, NQ, 3], f32)
            nc.scalar.dma_start(out=w2_t, in_=w2_d)
            b2_t = head.tile([3, 1], f32)
            nc.scalar.dma_start(out=b2_t, in_=b2_d)
            idt = singles.tile([BPC, BPC], f32)
            make_identity(nc, idt)

            # --- bert row chunks on the SP ring, grouped ---
            bt = singles.tile([128, totch * H], bdt)
            c0 = 0
            for g in groups:
                nc.sync.dma_start(out=bt[:, c0 * H:(c0 + g) * H],
                                  in_=bert_d[:, c0 * H:(c0 + g) * H])
                c0 += g

            # --- W1 on the ACT ring, after consts (needed only by mm2) ---
            w1p_t = singles.tile([128, HC, HID], wdt)
            for i in range(2):
                half = HC // 2
                nc.scalar.dma_start(out=w1p_t[:, half * i:half * (i + 1), :],
                                    in_=w1p_d[:, half * i:half * (i + 1), :])
            w1s_t = singles.tile([128, 2 * HC, HID], sdt)
            for i in range(2):
                nc.scalar.dma_start(out=w1s_t[:, HC * i:HC * (i + 1), :],
                                    in_=w1s_d[:, HC * i:HC * (i + 1), :])

            # xT split: span cols (e*BPC + slot, e in {A, B}) and pron
            xTs_t = singles.tile([128, HC, 2 * BPC], sdt)
            xTp_t = singles.tile([128, HC, BPC], wdt)

            # --- pron embedding: fp32 rows, transposed via PE ---
            for hc in range(HC):
                pxp = psum_p.tile([128, BPC], f32, tag="pxp")
                nc.tensor.transpose(pxp, pron_t[:, hc * 128:(hc + 1) * 128],
                                    idt)
                nc.vector.tensor_copy(xTp_t[:, hc, :], pxp)

            # --- mm1: span sums for all slots at once ---
            # The mask column encodes (span, slot), so chunks may mix
            # batches freely. PSUM accumulation groups must be closed
            # before the next opens (concurrent open groups in a bank
            # corrupt each other), so accumulate per DMA group in PSUM
            # and flush to an SBUF accumulator.
            xacc = singles.tile([128, HC, NMC], f32)
            hp_sb = singles.tile([128, NQ, BPC], f32)
            c0 = 0
            for gi, g in enumerate(groups):
                if gi == len(groups) - 1:
                    # pron half of mm2: xTp and w1p are ready by now, so
                    # this fills the PE while the last bert group lands.
                    for q in range(NQ):
                        phTp = psum_hp.tile([128, BPC], f32, tag="phTp")
                        for hc in range(HC):
                            nc.tensor.matmul(
                                phTp,
                                w1p_t[:, hc, q * 128:(q + 1) * 128],
                                xTp_t[:, hc, :],
                                start=(hc == 0), stop=(hc == HC - 1))
                        nc.vector.tensor_copy(hp_sb[:, q, :], phTp)
                for hc in range(HC):
                    pxg = psum_x.tile([128, NMC], f32, tag="px")
                    for j in range(g):
                        sc = c0 + j
                        nc.tensor.matmul(
                            pxg,
                            bt[:, sc * H + hc * 128:sc * H + (hc + 1) * 128],
                            mask_t[:, sc, :],
                            start=(j == 0),
                            stop=(j == g - 1),
                        )
                    if gi == 0:
                        nc.vector.tensor_copy(xacc[:, hc, :], pxg)
                    else:
                        nc.vector.tensor_add(xacc[:, hc, :],
                                             xacc[:, hc, :], pxg)
                c0 += g
            # fp32 scale by 1/span_len on the SBUF->SBUF copy
            for hc in range(HC):
                nc.vector.tensor_mul(xTs_t[:, hc, :], xacc[:, hc, :],
                                     sfac_t)

            # --- mm2: hT[q] = sum_kc W1sub.T @ xT chunk (24 k-chunks) ---
            # mm2 -> BN+LeakyReLU -> mm3, interleaved per hid quarter:
            # per-q PSUM tiles (rotating banks) let the DVE consume
            # quarter q while the PE runs quarter q+1. The span part
            # (scaled if fp8) and pron part accumulate separately and
            # recombine on the DVE; mm3 results accumulate in SBUF so
            # no PSUM group stays open across quarters.
            SPAN_UNSCALE = 1.0 / (64.0 * 32.0) if W1_SPAN_FP8 else 1.0
            o_acc = head.tile([3, BPC], f32)
            for q in range(NQ):
                phTs = psum_hs.tile([128, BPC], f32, tag="phTs")
                for kc in range(2 * HC):
                    e, hc = kc // HC, kc % HC
                    nc.tensor.matmul(
                        phTs,
                        w1s_t[:, kc, q * 128:(q + 1) * 128],
                        xTs_t[:, hc, e * BPC:(e + 1) * BPC],
                        start=(kc == 0), stop=(kc == 2 * HC - 1))
                t0_t = head.tile([128, BPC], f32, tag=f"t0_{q % 2}")
                nc.vector.tensor_scalar_add(t0_t, hp_sb[:, q, :],
                                            bnb_t[:, q:q + 1])
                t_t = head.tile([128, BPC], f32, tag=f"t_{q % 2}")
                nc.vector.scalar_tensor_tensor(
                    t_t, phTs, SPAN_UNSCALE, t0_t,
                    op0=mybir.AluOpType.mult, op1=mybir.AluOpType.add)
                y_t = head.tile([128, BPC], f32, tag=f"y_{q % 2}")
                # y = max(0.01 * t, t)
                nc.vector.scalar_tensor_tensor(
                    y_t, t_t, 0.01, t_t,
                    op0=mybir.AluOpType.mult, op1=mybir.AluOpType.max)
                pm3 = psum_p.tile([3, BPC], f32, tag="m3")
                nc.tensor.matmul(pm3, w2_t[:, q, :], y_t,
                                 start=True, stop=True)
                if q == 0:
                    nc.vector.tensor_scalar_add(o_acc, pm3, b2_t)
                else:
                    nc.vector.tensor_add(o_acc, o_acc, pm3)
            nc.sync.dma_start(out=out_d, in_=o_acc)

    nc.compile()
    return nc


def _prep_core_inputs(bert8, bert_f32, offsets, batch_idx, totch, np8):
    """Build the per-core input map for the given batch indices."""
    NMC = 2 * BPC
    bertp = np.zeros((totch, 128, H), dtype=bert8.dtype)
    sfac = np.ones((NMC,), dtype=np.float32)
    pron = np.empty((BPC, H), dtype=np.float32)
    flat_bert = bertp.reshape(totch * 128, H)
    flat_mask = np.zeros((totch * 128, NMC), dtype=bert8.dtype)
    rmw = np.asarray(1.0 / 64.0, dtype=np8)  # exact in fp8
    row = 0
    for slot, gb in enumerate(batch_idx):
        a0, a1, b0, b1_, p = (int(v) for v in offsets[gb])
        rows = np.union1d(np.arange(a0, a1 + 1), np.arange(b0, b1_ + 1))
        n = rows.shape[0]
        flat_bert[row:row + n] = bert8[gb, rows]
        flat_mask[row:row + n, 0 * BPC + slot] = \
            ((rows >= a0) & (rows <= a1)).astype(np.float32).astype(np8)
        flat_mask[row:row + n, 1 * BPC + slot] = \
            ((rows >= b0) & (rows <= b1_)).astype(np.float32).astype(np8)
        xsc = 32.0 if W1_SPAN_FP8 else 1.0
        sfac[0 * BPC + slot] = xsc / (a1 - a0 + 1)
        sfac[1 * BPC + slot] = xsc / (b1_ - b0 + 1)
        pron[slot] = bert_f32[gb, p]
        row += n
        if BERT_FP8:
            for e, (s0, s1) in enumerate(((a0, a1), (b0, b1_))):
                L = s1 - s0 + 1
                if L >= LTHR:
                    continue
                res = (bert_f32[gb, s0:s1 + 1]
                       - bert8[gb, s0:s1 + 1].astype(np.float32))
                flat_bert[row:row + L] = (res * 64.0).astype(np8)
                flat_mask[row:row + L, e * BPC + slot] = rmw
                row += L
    # partition-major layout: each SBUF partition line is contiguous DRAM
    maskp = np.ascontiguousarray(flat_mask.reshape(totch, 128, NMC)
                                 .transpose(1, 0, 2))
    return {
        "bertp": np.ascontiguousarray(
            bertp.transpose(1, 0, 2).reshape(128, totch * H)),
        "maskp": maskp,
        "sfac": np.broadcast_to(sfac, (128, NMC)).copy(),
        "pron": pron,
    }


def kernel(bert_outputs, offsets, W1, b1, gamma, beta, running_mean,
           running_var, W2, b2):
    import ml_dtypes

    np8 = ml_dtypes.float8_e4m3 if BERT_FP8 else ml_dtypes.bfloat16

    bert_f32 = np.ascontiguousarray(np.asarray(bert_outputs, dtype=np.float32))
    bert8 = bert_f32.astype(np8)
    offs = np.asarray(offsets).astype(np.int64)
    W1 = np.asarray(W1, dtype=np.float32)
    b1 = np.asarray(b1, dtype=np.float32)
    gamma = np.asarray(gamma, dtype=np.float32)
    beta = np.asarray(beta, dtype=np.float32)
    rm = np.asarray(running_mean, dtype=np.float32)
    rv = np.asarray(running_var, dtype=np.float32)
    W2 = np.asarray(W2, dtype=np.float32)
    b2 = np.asarray(b2, dtype=np.float32)

    # Fold BN eval-mode stats: bn(xW1 + b1) = x(W1*s) + ((b1 - mean)*s + beta)
    s = gamma / np.sqrt(rv + EPS)
    bias = (b1 - rm) * s + beta
    W1s = W1 * s[None, :]
    sdt_np = ml_dtypes.float8_e4m3 if W1_SPAN_FP8 else (
        ml_dtypes.bfloat16 if W1_BF16 else np.float32)
    wdt_np = ml_dtypes.bfloat16 if W1_BF16 else np.float32
    w1sP = np.ascontiguousarray(
        (W1s[:2 * H] * (64.0 if W1_SPAN_FP8 else 1.0))
        .reshape(2 * HC, 128, HID).transpose(1, 0, 2)
    ).astype(sdt_np)
    w1pP = np.ascontiguousarray(
        W1s[2 * H:].reshape(HC, 128, HID).transpose(1, 0, 2)
    ).astype(wdt_np)
    bnbP = np.ascontiguousarray(bias.reshape(NQ, 128).T)
    w2P = np.ascontiguousarray(W2.reshape(NQ, 128, 3).transpose(1, 0, 2))
    b2c = np.ascontiguousarray(b2.reshape(3, 1))

    # Exact union rows per batch; balance total rows across cores (LPT,
    # exactly BPC batches per core).
    nrows = np.empty(B, dtype=np.int64)
    for gb in range(B):
        a0, a1, b0, b1_, _ = (int(v) for v in offs[gb])
        # union size without materializing: overlap or disjoint
        if b0 <= a1 and a0 <= b1_:
            nrows[gb] = max(a1, b1_) - min(a0, b0) + 1
        else:
            nrows[gb] = (a1 - a0 + 1) + (b1_ - b0 + 1)
        if BERT_FP8:
            for s0, s1 in ((a0, a1), (b0, b1_)):
                if s1 - s0 + 1 < LTHR:
                    nrows[gb] += s1 - s0 + 1
    order = np.argsort(-nrows, kind="stable")
    loads = np.zeros(NCORES, dtype=np.int64)
    counts = np.zeros(NCORES, dtype=np.int64)
    asg = [[] for _ in range(NCORES)]
    for gb in order:
        open_cores = np.flatnonzero(counts < BPC)
        c = open_cores[np.argmin(loads[open_cores])]
        asg[c].append(int(gb))
        loads[c] += nrows[gb]
        counts[c] += 1
    totch = int((loads.max() + 127) // 128)

    if totch not in _PROGRAM_CACHE:
        _PROGRAM_CACHE[totch] = _build_program(totch)
    nc = _PROGRAM_CACHE[totch]

    shared = {"w1sP": w1sP, "w1pP": w1pP, "bnbP": bnbP, "w2P": w2P,
              "b2c": b2c}
    in_maps = []
    for c in range(NCORES):
        m = _prep_core_inputs(bert8, bert_f32, offs, asg[c], totch, np8)
        m.update(shared)
        in_maps.append(m)

    from concourse import bass_utils
    kwargs = {}
    if TRACE:
        kwargs = {"trace": True, "trace_cores": list(range(NCORES))}
    res = bass_utils.run_bass_kernel_spmd(nc, in_maps,
                                          core_ids=list(range(NCORES)),
                                          **kwargs)
    global LAST_RESULT
    LAST_RESULT = res

    out = np.empty((B, 3), dtype=np.float32)
    for c in range(NCORES):
        out[asg[c]] = res.results[c]["out"].T
    return out


# revision 37
# speedup vs baseline: 1.0090x; 1.0028x over previous
"""CorefHead Trainium2 kernel.

Reference computation (B=64, S=512, H=1024, HID=512):
  emb_a = span_mean(bert, offsets[:,0:2])   # [B,H]
  emb_b = span_mean(bert, offsets[:,2:4])   # [B,H]
  emb_p = bert[b, offsets[:,4]]             # [B,H]
  x = concat([emb_a, emb_b, emb_p], -1)     # [B,3H]
  h = leaky_relu(batchnorm_eval(x @ W1 + b1), 0.01)
  out = h @ W2 + b2                         # [B,3]

Strategy: pure data parallel, batch sharded 8 ways (8 batches/core),
DMA-volume minimized:
  - Host ships only the exact union rows (span A + span B) per batch,
    packed back-to-back across the core's 8 batches into 128-row chunks
    (chunks may cross batch boundaries). Rows are fp8-e4m3: span means
    average ~170 rows and the pron row dominates the final signal, so
    fp8 noise on span rows stays ~0.5% at the output. The pron rows ship
    separately in fp32 and are transposed on the PE.
  - mm1 (PE): per (DMA group, h-chunk) a PSUM tile [128, 16] accumulates
    bert_chunk.T @ mask_chunk, flushed into an SBUF accumulator by the
    DVE (PSUM accumulation groups must close before the next opens);
    the mask column encodes (span, slot) so batch identity lives in the
    mask and chunks may mix batches freely.
  - mm2 (PE, swapped operands): per hid quarter q, phT[q] +=
    W1sub[128k, 128hid].T @ xT[128k, 8] -> h transposed directly (no
    on-device transpose of h). Per-q rotating PSUM tiles + interleaved
    BN+LeakyReLU (DVE) and mm3 (PE, out[3,8] accumulated in SBUF) let
    the DVE consume quarter q while the PE runs quarter q+1.
  - DMA: bert rides the SP ring in ~0.5 MB groups (first group small to
    prime the mm1 pipeline); consts + W1 ride the ACT ring; W1 is only
    needed by mm2 at the end so bert is never stuck behind it.
Host gathers per-core [3, 8] outputs and undoes the batch permutation.
"""

import numpy as np

B, S, H = 64, 512, 1024
HID = 512
EPS = 1e-5
NCORES = 8
BPC = B // NCORES  # batches per core
KC = 3 * H // 128  # 24 contraction chunks for mm2
HC = H // 128      # 8 h-chunks per embedding
NQ = HID // 128    # 4 hid quarters

# bert span rows + masks in fp8-e4m3 (halves DMA vs bf16); pron fp32.
BERT_FP8 = True
# Spans shorter than this get a second pass of fp8 residual rows
# (v - fp8(v), same mask column): short spans don't average away fp8
# noise, and two fp8 levels beat bf16 precision for ~5% extra rows.
LTHR = 32
# W1 (and the mm2 xT operand) in bf16.
W1_BF16 = True
# W1 span blocks (rows 0..2047) + the span xT columns in fp8: their
# error contribution to h is attenuated ~10x because the pron block
# dominates h's variance. The pron block of W1 stays bf16.
W1_SPAN_FP8 = False

# Test-harness hooks (harness calls kernel() with TRACE=False default).
TRACE = False
LAST_RESULT = None

_PROGRAM_CACHE: dict = {}


def _bert_groups(totch: int):
    """Chunk-group sizes for the bert DMA: small first group to prime
    the mm1 pipeline, ~4-chunk (512 KB fp8) transfers in the middle,
    small last group so mm1 drains right at DMA end."""
    if totch <= 2:
        return [totch]
    groups = [2]
    left = totch - 2
    while left > 0:
        g = min(4, left)
        groups.append(g)
        left -= g
    if groups[-1] > 2:
        groups[-1] -= 1
        groups.append(1)
    return groups


def _build_program(totch: int):
    import concourse.bacc as bacc
    import concourse.tile as tile
    import concourse.mybir as mybir
    from concourse.bass import MemorySpace
    from concourse.masks import make_identity

    f32 = mybir.dt.float32
    bdt = mybir.dt.float8e4 if BERT_FP8 else mybir.dt.bfloat16
    wdt = mybir.dt.bfloat16 if W1_BF16 else f32
    sdt = mybir.dt.float8e4 if W1_SPAN_FP8 else wdt

    nc = bacc.Bacc("TRN2", target_bir_lowering=False, debug=False,
                   num_devices=NCORES)

    NMC = 2 * BPC  # mask columns: (span e, slot) -> e*BPC + slot

    bert_d = nc.dram_tensor("bertp", [128, totch * H], bdt,
                            kind="ExternalInput").ap()
    mask_d = nc.dram_tensor("maskp", [128, totch, NMC], bdt,
                            kind="ExternalInput").ap()
    sfac_d = nc.dram_tensor("sfac", [128, NMC], f32, kind="ExternalInput").ap()
    pron_d = nc.dram_tensor("pron", [BPC, H], f32, kind="ExternalInput").ap()
    w1s_d = nc.dram_tensor("w1sP", [128, 2 * HC, HID], sdt,
                           kind="ExternalInput").ap()
    w1p_d = nc.dram_tensor("w1pP", [128, HC, HID], wdt,
                           kind="ExternalInput").ap()
    bnb_d = nc.dram_tensor("bnbP", [128, NQ], f32, kind="ExternalInput").ap()
    w2_d = nc.dram_tensor("w2P", [128, NQ, 3], f32, kind="ExternalInput").ap()
    b2_d = nc.dram_tensor("b2c", [3, 1], f32, kind="ExternalInput").ap()
    out_d = nc.dram_tensor("out", [3, BPC], f32, kind="ExternalOutput").ap()

    groups = _bert_groups(totch)

    with tile.TileContext(nc) as tc:
        with (
            tc.tile_pool(name="singles", bufs=1) as singles,
            tc.tile_pool(name="head", bufs=1) as head,
            tc.tile_pool(name="psum_x", bufs=2, space=MemorySpace.PSUM) as psum_x,
            tc.tile_pool(name="psum_p", bufs=1, space=MemorySpace.PSUM) as psum_p,
            tc.tile_pool(name="psum_hs", bufs=2, space=MemorySpace.PSUM) as psum_hs,
            tc.tile_pool(name="psum_hp", bufs=2, space=MemorySpace.PSUM) as psum_hp,
        ):
            # --- consts on the ACT ring (tiny, land first) ---
            mask_t = singles.tile([128, totch, NMC], bdt)
            nc.scalar.dma_start(out=mask_t, in_=mask_d)
            sfac_t = singles.tile([128, NMC], f32)
            nc.scalar.dma_start(out=sfac_t, in_=sfac_d)
            pron_t = singles.tile([BPC, H], f32)
            nc.scalar.dma_start(out=pron_t, in_=pron_d)
            bnb_t = head.tile([128, NQ], f32)
            nc.scalar.dma_start(out=bnb_t, in_=bnb_d)
            w2_t = head.tile([128, NQ, 3], f32)
            nc.scalar.dma_start(out=w2_t, in_=w2_d)
            b2_t = head.tile([3, 1], f32)
            nc.scalar.dma_start(out=b2_t, in_=b2_d)
            idt = singles.tile([BPC, BPC], f32)
            make_identity(nc, idt)

            # --- bert row chunks on the SP ring, grouped ---
            bt = singles.tile([128, totch * H], bdt)
            c0 = 0
            for g in groups:
                nc.sync.dma_start(out=bt[:, c0 * H:(c0 + g) * H],
                                  in_=bert_d[:, c0 * H:(c0 + g) * H])
                c0 += g

            # --- W1 on the ACT ring, after consts (needed only by mm2) ---
            w1p_t = singles.tile([128, HC, HID], wdt)
            for i in range(2):
                half = HC // 2
                nc.scalar.dma_start(out=w1p_t[:, half * i:half * (i + 1), :],
                                    in_=w1p_d[:, half * i:half * (i + 1), :])
            w1s_t = singles.tile([128, 2 * HC, HID], sdt)
            for i in range(2):
                nc.scalar.dma_start(out=w1s_t[:, HC * i:HC * (i + 1), :],
                                    in_=w1s_d[:, HC * i:HC * (i + 1), :])

            # xT split: span cols (e*BPC + slot, e in {A, B}) and pron
            xTs_t = singles.tile([128, HC, 2 * BPC], sdt)
            xTp_t = singles.tile([128, HC, BPC], wdt)

            # --- pron embedding: fp32 rows, transposed via PE ---
            for hc in range(HC):
                pxp = psum_p.tile([128, BPC], f32, tag="pxp")
                nc.tensor.transpose(pxp, pron_t[:, hc * 128:(hc + 1) * 128],
                                    idt)
                nc.vector.tensor_copy(xTp_t[:, hc, :], pxp)

            # --- mm1: span sums for all slots at once ---
            # The mask column encodes (span, slot), so chunks may mix
            # batches freely. PSUM accumulation groups must be closed
            # before the next opens (concurrent open groups in a bank
            # corrupt each other), so accumulate per DMA group in PSUM
            # and flush to an SBUF accumulator.
            xacc = singles.tile([128, HC, NMC], f32)
            hp_sb = singles.tile([128, NQ, BPC], f32)
            c0 = 0
            for gi, g in enumerate(groups):
                if gi == len(groups) - 1:
                    # pron half of mm2: xTp and w1p are ready by now, so
                    # this fills the PE while the last bert group lands.
                    for q in range(NQ):
                        phTp = psum_hp.tile([128, BPC], f32, tag="phTp")
                        for hc in range(HC):
                            nc.tensor.matmul(
                                phTp,
                                w1p_t[:, hc, q * 128:(q + 1) * 128],
                                xTp_t[:, hc, :],
                                start=(hc == 0), stop=(hc == HC - 1))
                        nc.vector.tensor_copy(hp_sb[:, q, :], phTp)
                for hc in range(HC):
                    pxg = psum_x.tile([128, NMC], f32, tag="px")
                    for j in range(g):
                        sc = c0 + j
                        nc.tensor.matmul(
                            pxg,
                            bt[:, sc * H + hc * 128:sc * H + (hc + 1) * 128],
                            mask_t[:, sc, :],
                            start=(j == 0),
                            stop=(j == g - 1),
                        )
                    if gi == 0:
                        nc.vector.tensor_copy(xacc[:, hc, :], pxg)
                    else:
                        nc.vector.tensor_add(xacc[:, hc, :],
                                             xacc[:, hc, :], pxg)
                c0 += g
            # fp32 scale by 1/span_len on the SBUF->SBUF copy
            for hc in range(HC):
                nc.vector.tensor_mul(xTs_t[:, hc, :], xacc[:, hc, :],
                                     sfac_t)

            # --- mm2: hT[q] = sum_kc W1sub.T @ xT chunk (24 k-chunks) ---
            # mm2 -> BN+LeakyReLU -> mm3, interleaved per hid quarter:
            # per-q PSUM tiles (rotating banks) let the DVE consume
            # quarter q while the PE runs quarter q+1. The span part
            # (scaled if fp8) and pron part accumulate separately and
            # recombine on the DVE; mm3 results accumulate in SBUF so
            # no PSUM group stays open across quarters.
            SPAN_UNSCALE = 1.0 / (64.0 * 32.0) if W1_SPAN_FP8 else 1.0
            o_acc = head.tile([3, BPC], f32)
            for q in range(NQ):
                phTs = psum_hs.tile([128, BPC], f32, tag="phTs")
                for kc in range(2 * HC):
                    e, hc = kc // HC, kc % HC
                    nc.tensor.matmul(
                        phTs,
                        w1s_t[:, kc, q * 128:(q + 1) * 128],
                        xTs_t[:, hc, e * BPC:(e + 1) * BPC],
                        start=(kc == 0), stop=(kc == 2 * HC - 1))
                t0_t = head.tile([128, BPC], f32, tag=f"t0_{q % 2}")
                nc.vector.tensor_scalar_add(t0_t, hp_sb[:, q, :],
                                            bnb_t[:, q:q + 1])
                t_t = head.tile([128, BPC], f32, tag=f"t_{q % 2}")
                nc.vector.scalar_tensor_tensor(
                    t_t, phTs, SPAN_UNSCALE, t0_t,
                    op0=mybir.AluOpType.mult, op1=mybir.AluOpType.add)
                y_t = head.tile([128, BPC], f32, tag=f"y_{q % 2}")
                # y = max(0.01 * t, t)
                nc.vector.scalar_tensor_tensor(
                    y_t, t_t, 0.01, t_t,
                    op0=mybir.AluOpType.mult, op1=mybir.AluOpType.max)
                pm3 = psum_p.tile([3, BPC], f32, tag="m3")
                nc.tensor.matmul(pm3, w2_t[:, q, :], y_t,
                                 start=True, stop=True)
                if q == 0:
                    nc.vector.tensor_scalar_add(o_acc, pm3, b2_t)
                else:
                    nc.vector.tensor_add(o_acc, o_acc, pm3)
            nc.sync.dma_start(out=out_d, in_=o_acc)

    nc.compile()
    return nc


def _prep_core_inputs(bert8, bert_f32, offsets, batch_idx, totch, np8):
    """Build the per-core input map for the given batch indices."""
    NMC = 2 * BPC
    bertp = np.zeros((totch, 128, H), dtype=bert8.dtype)
    sfac = np.ones((NMC,), dtype=np.float32)
    pron = np.empty((BPC, H), dtype=np.float32)
    flat_bert = bertp.reshape(totch * 128, H)
    flat_mask = np.zeros((totch * 128, NMC), dtype=bert8.dtype)
    rmw = np.asarray(1.0 / 64.0, dtype=np8)  # exact in fp8
    row = 0
    for slot, gb in enumerate(batch_idx):
        a0, a1, b0, b1_, p = (int(v) for v in offsets[gb])
        rows = np.union1d(np.arange(a0, a1 + 1), np.arange(b0, b1_ + 1))
        n = rows.shape[0]
        flat_bert[row:row + n] = bert8[gb, rows]
        flat_mask[row:row + n, 0 * BPC + slot] = \
            ((rows >= a0) & (rows <= a1)).astype(np.float32).astype(np8)
        flat_mask[row:row + n, 1 * BPC + slot] = \
            ((rows >= b0) & (rows <= b1_)).astype(np.float32).astype(np8)
        xsc = 32.0 if W1_SPAN_FP8 else 1.0
        sfac[0 * BPC + slot] = xsc / (a1 - a0 + 1)
        sfac[1 * BPC + slot] = xsc / (b1_ - b0 + 1)
        pron[slot] = bert_f32[gb, p]
        row += n
        if BERT_FP8:
            for e, (s0, s1) in enumerate(((a0, a1), (b0, b1_))):
                L = s1 - s0 + 1
                if L >= LTHR:
                    continue
                res = (bert_f32[gb, s0:s1 + 1]
                       - bert8[gb, s0:s1 + 1].astype(np.float32))
                flat_bert[row:row + L] = (res * 64.0).astype(np8)
                flat_mask[row:row + L, e * BPC + slot] = rmw
                row += L
    # partition-major layout: each SBUF partition line is contiguous DRAM
    maskp = np.ascontiguousarray(flat_mask.reshape(totch, 128, NMC)
                                 .transpose(1, 0, 2))
    return {
        "bertp": np.ascontiguousarray(
            bertp.transpose(1, 0, 2).reshape(128, totch * H)),
        "maskp": maskp,
        "sfac": np.broadcast_to(sfac, (128, NMC)).copy(),
        "pron": pron,
    }


def kernel(bert_outputs, offsets, W1, b1, gamma, beta, running_mean,
           running_var, W2, b2):
    import ml_dtypes

    np8 = ml_dtypes.float8_e4m3 if BERT_FP8 else ml_dtypes.bfloat16

    bert_f32 = np.ascontiguousarray(np.asarray(bert_outputs, dtype=np.float32))
    bert8 = bert_f32.astype(np8)
    offs = np.asarray(offsets).astype(np.int64)
    W1 = np.asarray(W1, dtype=np.float32)
    b1 = np.asarray(b1, dtype=np.float32)
    gamma = np.asarray(gamma, dtype=np.float32)
    beta = np.asarray(beta, dtype=np.float32)
    rm = np.asarray(running_mean, dtype=np.float32)
    rv = np.asarray(running_var, dtype=np.float32)
    W2 = np.asarray(W2, dtype=np.float32)
    b2 = np.asarray(b2, dtype=np.float32)

    # Fold BN eval-mode stats: bn(xW1 + b1) = x(W1*s) + ((b1 - mean)*s + beta)
    s = gamma / np.sqrt(rv + EPS)
    bias = (b1 - rm) * s + beta
    W1s = W1 * s[None, :]
    sdt_np = ml_dtypes.float8_e4m3 if W1_SPAN_FP8 else (
        ml_dtypes.bfloat16 if W1_BF16 else np.float32)
    wdt_np = ml_dtypes.bfloat16 if W1_BF16 else np.float32
    w1sP = np.ascontiguousarray(
        (W1s[:2 * H] * (64.0 if W1_SPAN_FP8 else 1.0))
        .reshape(2 * HC, 128, HID).transpose(1, 0, 2)
    ).astype(sdt_np)
    w1pP = np.ascontiguousarray(
        W1s[2 * H:].reshape(HC, 128, HID).transpose(1, 0, 2)
    ).astype(wdt_np)
    bnbP = np.ascontiguousarray(bias.reshape(NQ, 128).T)
    w2P = np.ascontiguousarray(W2.reshape(NQ, 128, 3).transpose(1, 0, 2))
    b2c = np.ascontiguousarray(b2.reshape(3, 1))

    # Exact union rows per batch; balance total rows across cores (LPT,
    # exactly BPC batches per core).
    nrows = np.empty(B, dtype=np.int64)
    for gb in range(B):
        a0, a1, b0, b1_, _ = (int(v) for v in offs[gb])
        # union size without materializing: overlap or disjoint
        if b0 <= a1 and a0 <= b1_:
            nrows[gb] = max(a1, b1_) - min(a0, b0) + 1
        else:
            nrows[gb] = (a1 - a0 + 1) + (b1_ - b0 + 1)
        if BERT_FP8:
            for s0, s1 in ((a0, a1), (b0, b1_)):
                if s1 - s0 + 1 < LTHR:
                    nrows[gb] += s1 - s0 + 1
    order = np.argsort(-nrows, kind="stable")
    loads = np.zeros(NCORES, dtype=np.int64)
    counts = np.zeros(NCORES, dtype=np.int64)
    asg = [[] for _ in range(NCORES)]
    for gb in order:
        open_cores = np.flatnonzero(counts < BPC)
        c = open_cores[np.argmin(loads[open_cores])]
        asg[c].append(int(gb))
        loads[c] += nrows[gb]
        counts[c] += 1
    totch = int((loads.max() + 127) // 128)

    if totch not in _PROGRAM_CACHE:
        _PROGRAM_CACHE[totch] = _build_program(totch)
    nc = _PROGRAM_CACHE[totch]

    shared = {"w1sP": w1sP, "w1pP": w1pP, "bnbP": bnbP, "w2P": w2P,
              "b2c": b2c}
    in_maps = []
    for c in range(NCORES):
        m = _prep_core_inputs(bert8, bert_f32, offs, asg[c], totch, np8)
        m.update(shared)
        in_maps.append(m)

    from concourse import bass_utils
    kwargs = {}
    if TRACE:
        kwargs = {"trace": True, "trace_cores": list(range(NCORES))}
    res = bass_utils.run_bass_kernel_spmd(nc, in_maps,
                                          core_ids=list(range(NCORES)),
                                          **kwargs)
    global LAST_RESULT
    LAST_RESULT = res

    out = np.empty((B, 3), dtype=np.float32)
    for c in range(NCORES):
        out[asg[c]] = res.results[c]["out"].T
    return out


# revision 38
# speedup vs baseline: 1.0173x; 1.0082x over previous
"""CorefHead Trainium2 kernel.

Reference computation (B=64, S=512, H=1024, HID=512):
  emb_a = span_mean(bert, offsets[:,0:2])   # [B,H]
  emb_b = span_mean(bert, offsets[:,2:4])   # [B,H]
  emb_p = bert[b, offsets[:,4]]             # [B,H]
  x = concat([emb_a, emb_b, emb_p], -1)     # [B,3H]
  h = leaky_relu(batchnorm_eval(x @ W1 + b1), 0.01)
  out = h @ W2 + b2                         # [B,3]

Strategy: pure data parallel, batch sharded 8 ways (8 batches/core),
DMA-volume minimized:
  - Host ships only the exact union rows (span A + span B) per batch,
    packed back-to-back across the core's 8 batches into 128-row chunks
    (chunks may cross batch boundaries). Rows are fp8-e4m3: span means
    average ~170 rows and the pron row dominates the final signal, so
    fp8 noise on span rows stays ~0.5% at the output. The pron rows ship
    separately in fp32 and are transposed on the PE.
  - mm1 (PE): per (DMA group, h-chunk) a PSUM tile [128, 16] accumulates
    bert_chunk.T @ mask_chunk, flushed into an SBUF accumulator by the
    DVE (PSUM accumulation groups must close before the next opens);
    the mask column encodes (span, slot) so batch identity lives in the
    mask and chunks may mix batches freely.
  - mm2 (PE, swapped operands): per hid quarter q, phT[q] +=
    W1sub[128k, 128hid].T @ xT[128k, 8] -> h transposed directly (no
    on-device transpose of h). Per-q rotating PSUM tiles + interleaved
    BN+LeakyReLU (DVE) and mm3 (PE, out[3,8] accumulated in SBUF) let
    the DVE consume quarter q while the PE runs quarter q+1.
  - DMA: bert rides the SP ring in ~0.5 MB groups (first group small to
    prime the mm1 pipeline); consts + W1 ride the ACT ring; W1 is only
    needed by mm2 at the end so bert is never stuck behind it.
Host gathers per-core [3, 8] outputs and undoes the batch permutation.
"""

import numpy as np

B, S, H = 64, 512, 1024
HID = 512
EPS = 1e-5
NCORES = 8
BPC = B // NCORES  # batches per core
KC = 3 * H // 128  # 24 contraction chunks for mm2
HC = H // 128      # 8 h-chunks per embedding
NQ = HID // 128    # 4 hid quarters

# bert span rows + masks in fp8-e4m3 (halves DMA vs bf16); pron fp32.
BERT_FP8 = True
# Spans shorter than this get a second pass of fp8 residual rows
# (v - fp8(v), same mask column): short spans don't average away fp8
# noise, and two fp8 levels beat bf16 precision for ~5% extra rows.
LTHR = 32
# W1 (and the mm2 xT operand) in bf16.
W1_BF16 = True
# W1 span blocks (rows 0..2047) + the span xT columns in fp8: their
# error contribution to h is attenuated ~10x because the pron block
# dominates h's variance. The pron block of W1 stays bf16.
W1_SPAN_FP8 = False

# Test-harness hooks (harness calls kernel() with TRACE=False default).
TRACE = False
LAST_RESULT = None

_PROGRAM_CACHE: dict = {}


def _bert_groups(totch: int):
    """Chunk-group sizes for the bert DMA: small first group to prime
    the mm1 pipeline, ~4-chunk (512 KB fp8) transfers in the middle,
    small last group so mm1 drains right at DMA end."""
    if totch <= 2:
        return [totch]
    groups = [2]
    left = totch - 2
    while left > 0:
        g = min(4, left)
        groups.append(g)
        left -= g
    if groups[-1] > 2:
        groups[-1] -= 1
        groups.append(1)
    return groups


def _build_program(totch: int):
    import concourse.bacc as bacc
    import concourse.tile as tile
    import concourse.mybir as mybir
    from concourse.bass import MemorySpace
    from concourse.masks import make_identity

    f32 = mybir.dt.float32
    bdt = mybir.dt.float8e4 if BERT_FP8 else mybir.dt.bfloat16
    wdt = mybir.dt.bfloat16 if W1_BF16 else f32
    sdt = mybir.dt.float8e4 if W1_SPAN_FP8 else wdt

    nc = bacc.Bacc("TRN2", target_bir_lowering=False, debug=False,
                   num_devices=NCORES)

    NMC = 2 * BPC  # mask columns: (span e, slot) -> e*BPC + slot

    bert_d = nc.dram_tensor("bertp", [128, totch * H], bdt,
                            kind="ExternalInput").ap()
    mask_d = nc.dram_tensor("maskp", [128, totch, NMC], bdt,
                            kind="ExternalInput").ap()
    sfac_d = nc.dram_tensor("sfac", [128, NMC], f32, kind="ExternalInput").ap()
    pron_d = nc.dram_tensor("pron", [BPC, H], f32, kind="ExternalInput").ap()
    w1s_d = nc.dram_tensor("w1sP", [128, 2 * HC, HID], sdt,
                           kind="ExternalInput").ap()
    w1p_d = nc.dram_tensor("w1pP", [128, HC, HID], wdt,
                           kind="ExternalInput").ap()
    bnb_d = nc.dram_tensor("bnbP", [128, NQ], f32, kind="ExternalInput").ap()
    w2_d = nc.dram_tensor("w2P", [128, NQ, 3], f32, kind="ExternalInput").ap()
    b2_d = nc.dram_tensor("b2c", [3, 1], f32, kind="ExternalInput").ap()
    out_d = nc.dram_tensor("out", [3, BPC], f32, kind="ExternalOutput").ap()

    groups = _bert_groups(totch)

    with tile.TileContext(nc) as tc:
        with (
            tc.tile_pool(name="singles", bufs=1) as singles,
            tc.tile_pool(name="head", bufs=1) as head,
            tc.tile_pool(name="psum_x", bufs=2, space=MemorySpace.PSUM) as psum_x,
            tc.tile_pool(name="psum_p", bufs=1, space=MemorySpace.PSUM) as psum_p,
            tc.tile_pool(name="psum_hs", bufs=2, space=MemorySpace.PSUM) as psum_hs,
            tc.tile_pool(name="psum_hp", bufs=2, space=MemorySpace.PSUM) as psum_hp,
        ):
            # --- consts on the ACT ring (tiny, land first) ---
            mask_t = singles.tile([128, totch, NMC], bdt)
            nc.scalar.dma_start(out=mask_t, in_=mask_d)
            sfac_t = singles.tile([128, NMC], f32)
            nc.scalar.dma_start(out=sfac_t, in_=sfac_d)
            pron_t = singles.tile([BPC, H], f32)
            nc.scalar.dma_start(out=pron_t, in_=pron_d)
            bnb_t = head.tile([128, NQ], f32)
            nc.scalar.dma_start(out=bnb_t, in_=bnb_d)
            w2_t = head.tile([128, NQ, 3], f32)
            nc.scalar.dma_start(out=w2_t, in_=w2_d)
            b2_t = head.tile([3, 1], f32)
            nc.scalar.dma_start(out=b2_t, in_=b2_d)
            idt = singles.tile([BPC, BPC], f32)
            make_identity(nc, idt)

            # --- bert row chunks on the SP ring, grouped ---
            bt = singles.tile([128, totch * H], bdt)
            c0 = 0
            for g in groups:
                nc.sync.dma_start(out=bt[:, c0 * H:(c0 + g) * H],
                                  in_=bert_d[:, c0 * H:(c0 + g) * H])
                c0 += g

            # --- W1 on the ACT ring, after consts (needed only by mm2) ---
            w1p_t = singles.tile([128, HC, HID], wdt)
            nc.scalar.dma_start(out=w1p_t, in_=w1p_d)
            w1s_t = singles.tile([128, 2 * HC, HID], sdt)
            nc.scalar.dma_start(out=w1s_t, in_=w1s_d)

            # xT split: span cols (e*BPC + slot, e in {A, B}) and pron
            xTs_t = singles.tile([128, HC, 2 * BPC], sdt)
            xTp_t = singles.tile([128, HC, BPC], wdt)

            # --- pron embedding: fp32 rows, transposed via PE ---
            for hc in range(HC):
                pxp = psum_p.tile([128, BPC], f32, tag="pxp")
                nc.tensor.transpose(pxp, pron_t[:, hc * 128:(hc + 1) * 128],
                                    idt)
                nc.vector.tensor_copy(xTp_t[:, hc, :], pxp)

            # --- mm1: span sums for all slots at once ---
            # The mask column encodes (span, slot), so chunks may mix
            # batches freely. PSUM accumulation groups must be closed
            # before the next opens (concurrent open groups in a bank
            # corrupt each other), so accumulate per DMA group in PSUM
            # and flush to an SBUF accumulator.
            xacc = singles.tile([128, HC, NMC], f32)
            hp_sb = singles.tile([128, NQ, BPC], f32)
            c0 = 0
            for gi, g in enumerate(groups):
                if gi == len(groups) - 1:
                    # pron half of mm2: xTp and w1p are ready by now, so
                    # this fills the PE while the last bert group lands.
                    for q in range(NQ):
                        phTp = psum_hp.tile([128, BPC], f32, tag="phTp")
                        for hc in range(HC):
                            nc.tensor.matmul(
                                phTp,
                                w1p_t[:, hc, q * 128:(q + 1) * 128],
                                xTp_t[:, hc, :],
                                start=(hc == 0), stop=(hc == HC - 1))
                        nc.vector.tensor_copy(hp_sb[:, q, :], phTp)
                for hc in range(HC):
                    pxg = psum_x.tile([128, NMC], f32, tag="px")
                    for j in range(g):
                        sc = c0 + j
                        nc.tensor.matmul(
                            pxg,
                            bt[:, sc * H + hc * 128:sc * H + (hc + 1) * 128],
                            mask_t[:, sc, :],
                            start=(j == 0),
                            stop=(j == g - 1),
                        )
                    if gi == 0:
                        nc.vector.tensor_copy(xacc[:, hc, :], pxg)
                    else:
                        nc.vector.tensor_add(xacc[:, hc, :],
                                             xacc[:, hc, :], pxg)
                c0 += g
            # fp32 scale by 1/span_len on the SBUF->SBUF copy
            for hc in range(HC):
                nc.vector.tensor_mul(xTs_t[:, hc, :], xacc[:, hc, :],
                                     sfac_t)

            # --- mm2: hT[q] = sum_kc W1sub.T @ xT chunk (24 k-chunks) ---
            # mm2 -> BN+LeakyReLU -> mm3, interleaved per hid quarter:
            # per-q PSUM tiles (rotating banks) let the DVE consume
            # quarter q while the PE runs quarter q+1. The span part
            # (scaled if fp8) and pron part accumulate separately and
            # recombine on the DVE; mm3 results accumulate in SBUF so
            # no PSUM group stays open across quarters.
            SPAN_UNSCALE = 1.0 / (64.0 * 32.0) if W1_SPAN_FP8 else 1.0
            o_acc = head.tile([3, BPC], f32)
            for q in range(NQ):
                phTs = psum_hs.tile([128, BPC], f32, tag="phTs")
                for kc in range(2 * HC):
                    e, hc = kc // HC, kc % HC
                    nc.tensor.matmul(
                        phTs,
                        w1s_t[:, kc, q * 128:(q + 1) * 128],
                        xTs_t[:, hc, e * BPC:(e + 1) * BPC],
                        start=(kc == 0), stop=(kc == 2 * HC - 1))
                t0_t = head.tile([128, BPC], f32, tag=f"t0_{q % 2}")
                nc.vector.tensor_scalar_add(t0_t, hp_sb[:, q, :],
                                            bnb_t[:, q:q + 1])
                t_t = head.tile([128, BPC], f32, tag=f"t_{q % 2}")
                nc.vector.scalar_tensor_tensor(
                    t_t, phTs, SPAN_UNSCALE, t0_t,
                    op0=mybir.AluOpType.mult, op1=mybir.AluOpType.add)
                y_t = head.tile([128, BPC], f32, tag=f"y_{q % 2}")
                # y = max(0.01 * t, t)
                nc.vector.scalar_tensor_tensor(
                    y_t, t_t, 0.01, t_t,
                    op0=mybir.AluOpType.mult, op1=mybir.AluOpType.max)
                pm3 = psum_p.tile([3, BPC], f32, tag="m3")
                nc.tensor.matmul(pm3, w2_t[:, q, :], y_t,
                                 start=True, stop=True)
                if q == 0:
                    nc.vector.tensor_scalar_add(o_acc, pm3, b2_t)
                else:
                    nc.vector.tensor_add(o_acc, o_acc, pm3)
            nc.sync.dma_start(out=out_d, in_=o_acc)

    nc.compile()
    return nc


def _prep_core_inputs(bert8, bert_f32, offsets, batch_idx, totch, np8):
    """Build the per-core input map for the given batch indices."""
    NMC = 2 * BPC
    bertp = np.zeros((totch, 128, H), dtype=bert8.dtype)
    sfac = np.ones((NMC,), dtype=np.float32)
    pron = np.empty((BPC, H), dtype=np.float32)
    flat_bert = bertp.reshape(totch * 128, H)
    flat_mask = np.zeros((totch * 128, NMC), dtype=bert8.dtype)
    rmw = np.asarray(1.0 / 64.0, dtype=np8)  # exact in fp8
    row = 0
    for slot, gb in enumerate(batch_idx):
        a0, a1, b0, b1_, p = (int(v) for v in offsets[gb])
        rows = np.union1d(np.arange(a0, a1 + 1), np.arange(b0, b1_ + 1))
        n = rows.shape[0]
        flat_bert[row:row + n] = bert8[gb, rows]
        flat_mask[row:row + n, 0 * BPC + slot] = \
            ((rows >= a0) & (rows <= a1)).astype(np.float32).astype(np8)
        flat_mask[row:row + n, 1 * BPC + slot] = \
            ((rows >= b0) & (rows <= b1_)).astype(np.float32).astype(np8)
        xsc = 32.0 if W1_SPAN_FP8 else 1.0
        sfac[0 * BPC + slot] = xsc / (a1 - a0 + 1)
        sfac[1 * BPC + slot] = xsc / (b1_ - b0 + 1)
        pron[slot] = bert_f32[gb, p]
        row += n
        if BERT_FP8:
            for e, (s0, s1) in enumerate(((a0, a1), (b0, b1_))):
                L = s1 - s0 + 1
                if L >= LTHR:
                    continue
                res = (bert_f32[gb, s0:s1 + 1]
                       - bert8[gb, s0:s1 + 1].astype(np.float32))
                flat_bert[row:row + L] = (res * 64.0).astype(np8)
                flat_mask[row:row + L, e * BPC + slot] = rmw
                row += L
    # partition-major layout: each SBUF partition line is contiguous DRAM
    maskp = np.ascontiguousarray(flat_mask.reshape(totch, 128, NMC)
                                 .transpose(1, 0, 2))
    return {
        "bertp": np.ascontiguousarray(
            bertp.transpose(1, 0, 2).reshape(128, totch * H)),
        "maskp": maskp,
        "sfac": np.broadcast_to(sfac, (128, NMC)).copy(),
        "pron": pron,
    }


def kernel(bert_outputs, offsets, W1, b1, gamma, beta, running_mean,
           running_var, W2, b2):
    import ml_dtypes

    np8 = ml_dtypes.float8_e4m3 if BERT_FP8 else ml_dtypes.bfloat16

    bert_f32 = np.ascontiguousarray(np.asarray(bert_outputs, dtype=np.float32))
    bert8 = bert_f32.astype(np8)
    offs = np.asarray(offsets).astype(np.int64)
    W1 = np.asarray(W1, dtype=np.float32)
    b1 = np.asarray(b1, dtype=np.float32)
    gamma = np.asarray(gamma, dtype=np.float32)
    beta = np.asarray(beta, dtype=np.float32)
    rm = np.asarray(running_mean, dtype=np.float32)
    rv = np.asarray(running_var, dtype=np.float32)
    W2 = np.asarray(W2, dtype=np.float32)
    b2 = np.asarray(b2, dtype=np.float32)

    # Fold BN eval-mode stats: bn(xW1 + b1) = x(W1*s) + ((b1 - mean)*s + beta)
    s = gamma / np.sqrt(rv + EPS)
    bias = (b1 - rm) * s + beta
    W1s = W1 * s[None, :]
    sdt_np = ml_dtypes.float8_e4m3 if W1_SPAN_FP8 else (
        ml_dtypes.bfloat16 if W1_BF16 else np.float32)
    wdt_np = ml_dtypes.bfloat16 if W1_BF16 else np.float32
    w1sP = np.ascontiguousarray(
        (W1s[:2 * H] * (64.0 if W1_SPAN_FP8 else 1.0))
        .reshape(2 * HC, 128, HID).transpose(1, 0, 2)
    ).astype(sdt_np)
    w1pP = np.ascontiguousarray(
        W1s[2 * H:].reshape(HC, 128, HID).transpose(1, 0, 2)
    ).astype(wdt_np)
    bnbP = np.ascontiguousarray(bias.reshape(NQ, 128).T)
    w2P = np.ascontiguousarray(W2.reshape(NQ, 128, 3).transpose(1, 0, 2))
    b2c = np.ascontiguousarray(b2.reshape(3, 1))

    # Exact union rows per batch; balance total rows across cores (LPT,
    # exactly BPC batches per core).
    nrows = np.empty(B, dtype=np.int64)
    for gb in range(B):
        a0, a1, b0, b1_, _ = (int(v) for v in offs[gb])
        # union size without materializing: overlap or disjoint
        if b0 <= a1 and a0 <= b1_:
            nrows[gb] = max(a1, b1_) - min(a0, b0) + 1
        else:
            nrows[gb] = (a1 - a0 + 1) + (b1_ - b0 + 1)
        if BERT_FP8:
            for s0, s1 in ((a0, a1), (b0, b1_)):
                if s1 - s0 + 1 < LTHR:
                    nrows[gb] += s1 - s0 + 1
    order = np.argsort(-nrows, kind="stable")
    loads = np.zeros(NCORES, dtype=np.int64)
    counts = np.zeros(NCORES, dtype=np.int64)
    asg = [[] for _ in range(NCORES)]
    for gb in order:
        open_cores = np.flatnonzero(counts < BPC)
        c = open_cores[np.argmin(loads[open_cores])]
        asg[c].append(int(gb))
        loads[c] += nrows[gb]
        counts[c] += 1
    totch = int((loads.max() + 127) // 128)

    if totch not in _PROGRAM_CACHE:
        _PROGRAM_CACHE[totch] = _build_program(totch)
    nc = _PROGRAM_CACHE[totch]

    shared = {"w1sP": w1sP, "w1pP": w1pP, "bnbP": bnbP, "w2P": w2P,
              "b2c": b2c}
    in_maps = []
    for c in range(NCORES):
        m = _prep_core_inputs(bert8, bert_f32, offs, asg[c], totch, np8)
        m.update(shared)
        in_maps.append(m)

    from concourse import bass_utils
    kwargs = {}
    if TRACE:
        kwargs = {"trace": True, "trace_cores": list(range(NCORES))}
    res = bass_utils.run_bass_kernel_spmd(nc, in_maps,
                                          core_ids=list(range(NCORES)),
                                          **kwargs)
    global LAST_RESULT
    LAST_RESULT = res

    out = np.empty((B, 3), dtype=np.float32)
    for c in range(NCORES):
        out[asg[c]] = res.results[c]["out"].T
    return out
